# revision 17
# baseline (speedup 1.0000x reference)
"""GAT-style graph encoder on 8 trn2 NeuronCores.

Reference computation (per exercise row i over kc nodes j):
    kc_Wh = kc_h @ W1; ex_Wh = ex_h @ W1
    e[i,j] = leaky_relu(ex_Wh[i]@a1 + kc_Wh[j]@a2, 0.2)
    att = softmax(where(adj>0, e, -9e15), axis=1)
    new_kc = att @ kc_Wh; ex_Eh = ex_h @ E
    out = elu(concat([new_kc, new_kc*ex_Eh]) @ rd_w.T + rd_b)

Strategy: row-shard exercises over 8 cores (1250 rows each, padded to 1280).
On-chip everything lives in a transposed [kc_or_feature, exercise] layout so
softmax numerator/denominator are plain PE matmuls contracting over the kc
partition axis -- no on-chip transposes.  Masking is a multiply (adj is 0/1)
on the exp'd logits; since logits are bounded (|e| <~ 15) the softmax is
computed without max-subtraction, exactly matching reference semantics to
f32 roundoff.  ex_a1 enters via the per-partition broadcast tile, kc_a2 via
the activation bias port, both folded through W1 on the host (weight-only
algebra: ex_Wh@a1 == ex_h@(W1@a1)).
"""

import ml_dtypes
import numpy as np

import concourse.bacc as bacc
import concourse.bass as bass
import concourse.mybir as mybir
from concourse.alu_op_type import AluOpType
from concourse.bass_utils import run_bass_kernel_spmd
from concourse.tile import TileContext

F32 = mybir.dt.float32
F32R = mybir.dt.float32r
BF16 = mybir.dt.bfloat16
AF = mybir.ActivationFunctionType

P = 128
D = 256                    # feature dim
NKC = 2048                 # padded kc count (2000 real)
KCH = NKC // P             # 16 kc chunks
M = 1280                   # padded exercise rows per core (1250 real)
MBS = (512, 512, 256)      # m blocks (>=256 keeps float32r at 1 cyc/row)
MOFF = (0, 512, 1024)
NCORES = 8
ROWS = 1250
N_E = 10000
ALPHA = 0.2
# A: 0/1 multiply-mask (ACT leaky+exp, DVE mask)
# B: fold, Pool tt, ACT leaky | C: fold, DVE tt, ACT leaky
# D: fold, Pool tt, DVE leaky | E: fold, DVE tt, DVE leaky
VARIANTS = ("B", "E", "A", "D", "B", "C", "A", "D")


def _build():
    nc = bacc.Bacc("TRN2", target_bir_lowering=False, debug=False,
                   num_devices=NCORES)
    exT = nc.declare_dram_parameter("exT", [2 * P, M], F32R, isOutput=False)
    adjT = nc.declare_dram_parameter("adjT", [NKC, M], BF16, isOutput=False)
    kcT = nc.declare_dram_parameter("kcT", [2 * P, NKC], F32R, isOutput=False)
    W1e = nc.declare_dram_parameter("W1e", [2 * P, D + 2], F32R, isOutput=False)
    w1a1 = nc.declare_dram_parameter("w1a1", [2 * P, 1], F32R, isOutput=False)
    Em = nc.declare_dram_parameter("Em", [2 * P, D], F32R, isOutput=False)
    rdwT = nc.declare_dram_parameter("rdwT", [4 * P, D], F32R, isOutput=False)
    rdb = nc.declare_dram_parameter("rdb", [2 * P, 1], F32, isOutput=False)
    outT = nc.declare_dram_parameter("outT", [2 * P, M], F32, isOutput=True)

    with TileContext(nc) as tc:
        with tc.tile_pool(name="const", bufs=1) as cpool, \
             tc.tile_pool(name="acc_ps", bufs=1, space="PSUM") as apool, \
             tc.tile_pool(name="out_ps", bufs=1, space="PSUM") as opool, \
             tc.tile_pool(name="mwork", bufs=8) as mpool, \
             tc.tile_pool(name="post", bufs=2) as qpool:
            kcT_sb, W1e_sb, Em_sb, w1a1_sb, rdb_sb, exT_sb = [], [], [], [], [], []
            for c in range(2):
                t = cpool.tile([P, NKC], F32R, tag=f"kcT{c}")
                nc.sync.dma_start(out=t[:], in_=kcT[c * P:(c + 1) * P, :])
                kcT_sb.append(t)
                t = cpool.tile([P, D + 2], F32R, tag=f"W1e{c}")
                nc.sync.dma_start(out=t[:], in_=W1e[c * P:(c + 1) * P, :])
                W1e_sb.append(t)
                t = cpool.tile([P, D], F32R, tag=f"Em{c}")
                nc.sync.dma_start(out=t[:], in_=Em[c * P:(c + 1) * P, :])
                Em_sb.append(t)
                t = cpool.tile([P, 1], F32R, tag=f"w1a1{c}")
                nc.sync.dma_start(out=t[:], in_=w1a1[c * P:(c + 1) * P, :])
                w1a1_sb.append(t)
                t = cpool.tile([P, 1], F32, tag=f"rdb{c}")
                nc.sync.dma_start(out=t[:], in_=rdb[c * P:(c + 1) * P, :])
                rdb_sb.append(t)
                t = cpool.tile([P, M], F32R, tag=f"exT{c}")
                nc.sync.dma_start(out=t[:], in_=exT[c * P:(c + 1) * P, :])
                exT_sb.append(t)
            rdwT_sb = []
            for dd in range(4):
                t = cpool.tile([P, D], F32R, tag=f"rdwT{dd}")
                nc.sync.dma_start(out=t[:], in_=rdwT[dd * P:(dd + 1) * P, :])
                rdwT_sb.append(t)
            ones1f = cpool.tile([1, P], F32, tag="ones1f")
            nc.vector.memset(ones1f[:], 1.0)
            ones1 = cpool.tile([1, P], F32R, tag="ones1")
            nc.scalar.copy(ones1[:], ones1f[:])
            ones128f = cpool.tile([P, 1], F32, tag="ones128f")
            nc.vector.memset(ones128f[:], 1.0)
            ones128 = cpool.tile([P, 1], F32R, tag="ones128")
            nc.scalar.copy(ones128[:], ones128f[:])

            # ---- setup (emitted in dependency-criticality order:
            # exa1b gates every main-loop block, kcWh[kk] gates chunk kk,
            # exEhT is needed only at the post stage of block 0)
            kcWh, kca2 = [], []
            exa1b = cpool.tile([P, M], F32, tag="exa1b")
            exa1_sb = cpool.tile([1, M], F32R, tag="exa1_sb")
            exEhT = [cpool.tile([P, M], F32, tag=f"exEhT{d}", name=f"exEhT{d}")
                     for d in range(2)]
            with tc.tile_pool(name="setup_ps", bufs=2, space="PSUM") as spool:
                for b in range(3):
                    ms = slice(MOFF[b], MOFF[b] + MBS[b])
                    ps = spool.tile([1, MBS[b]], F32, tag="misc_ps",
                                    name=f"row_ps{b}")
                    for c in range(2):
                        nc.tensor.matmul(ps[:], w1a1_sb[c][:],
                                         exT_sb[c][:, ms],
                                         start=(c == 0), stop=(c == 1))
                    nc.vector.tensor_copy(exa1_sb[:, ms], ps[:])
                    psb = spool.tile([P, MBS[b]], F32, tag="misc_ps",
                                     name=f"bc_ps{b}")
                    nc.tensor.matmul(psb[:], ones1[:], exa1_sb[:, ms],
                                     start=True, stop=True)
                    nc.vector.tensor_copy(exa1b[:, ms], psb[:])
                for kk in range(KCH):
                    ps = spool.tile([P, D + 2], F32, tag="kcwh_ps")
                    for c in range(2):
                        nc.tensor.matmul(
                            ps[:], kcT_sb[c][:, kk * P:(kk + 1) * P],
                            W1e_sb[c][:], start=(c == 0), stop=(c == 1))
                    t = cpool.tile([P, D], F32R, tag=f"kcWh{kk}",
                                   name=f"kcWh{kk}")
                    eng = nc.scalar if kk % 2 == 0 else nc.vector
                    if kk % 2 == 0:
                        nc.scalar.copy(t[:], ps[:, 0:D])
                    else:
                        nc.vector.tensor_copy(t[:], ps[:, 0:D])
                    kcWh.append(t)
                    tb = cpool.tile([P, 1], F32, tag=f"kca2_{kk}",
                                    name=f"kca2_{kk}")
                    nc.scalar.copy(tb[:], ps[:, D:D + 1])
                    kca2.append(tb)
                for d in range(2):
                    for b in range(3):
                        ms = slice(MOFF[b], MOFF[b] + MBS[b])
                        pse = spool.tile([P, MBS[b]], F32, tag="misc_ps",
                                         name=f"eh_ps{b}_{d}")
                        for c in range(2):
                            nc.tensor.matmul(
                                pse[:], Em_sb[c][:, d * P:(d + 1) * P],
                                exT_sb[c][:, ms], start=(c == 0), stop=(c == 1))
                        nc.scalar.copy(exEhT[d][:, ms], pse[:])

            # ---- main: masked softmax attention + aggregation + readout.
            # adjT row encoding is per-chunk (host-matched): chunks with
            # kk % 4 == 2 carry adj as 0/1 (multiply mask); all others carry
            # 100*(adj-1), i.e. 0 / -100, folded into the logits so that
            # leaky(-100+s) -> exp ~ 2e-9 ~ 0.
            for b in range(3):
                mb = MBS[b]
                ms = slice(MOFF[b], MOFF[b] + mb)
                n0 = apool.tile([P, mb], F32, tag="n0")
                n1 = apool.tile([P, mb], F32, tag="n1")
                sS = apool.tile([1, mb], F32, tag="sS")
                for kk in range(KCH):
                    adjf = mpool.tile([P, mb], BF16, tag="adjf", bufs=12)
                    nc.sync.dma_start(out=adjf[:],
                                      in_=adjT[kk * P:(kk + 1) * P, ms])
                    # 8-chunk rotation balancing ACT/DVE/Pool; see VARIANTS
                    v = VARIANTS[kk % 8]
                    ptm = mpool.tile([P, mb], F32R, tag="ptm")
                    if v == "A":  # multiply-mask: leaky+exp ACT, mask DVE
                        et = mpool.tile([P, mb], F32, tag="et")
                        nc.scalar.activation(et[:], exa1b[:, ms], AF.Prelu,
                                             bias=kca2[kk][:], alpha=ALPHA)
                        pt = mpool.tile([P, mb], F32, tag="pt")
                        nc.scalar.activation(pt[:], et[:], AF.Exp)
                        nc.vector.tensor_mul(ptm[:], pt[:], adjf[:])
                    else:         # logit-fold variants
                        tt_eng = nc.gpsimd if v in ("B", "D") else nc.vector
                        tmp = mpool.tile([P, mb], F32, tag="tmp")
                        tt_eng.tensor_add(tmp[:], adjf[:], exa1b[:, ms])
                        et = mpool.tile([P, mb], F32, tag="et")
                        if v in ("B", "C"):   # leaky on ACT
                            nc.scalar.activation(et[:], tmp[:], AF.Prelu,
                                                 bias=kca2[kk][:], alpha=ALPHA)
                        else:                 # leaky on DVE
                            s02 = mpool.tile([P, mb], F32, tag="s02")
                            nc.vector.tensor_scalar(
                                s02[:], tmp[:], kca2[kk][:], ALPHA,
                                AluOpType.add, AluOpType.mult)
                            nc.vector.scalar_tensor_tensor(
                                et[:], tmp[:], kca2[kk][:], s02[:],
                                AluOpType.add, AluOpType.max)
                        nc.scalar.activation(ptm[:], et[:], AF.Exp)
                    st, sp = (kk == 0), (kk == KCH - 1)
                    nc.tensor.matmul(n0[:], kcWh[kk][:, 0:P], ptm[:],
                                     start=st, stop=sp)
                    nc.tensor.matmul(n1[:], kcWh[kk][:, P:2 * P], ptm[:],
                                     start=st, stop=sp)
                    nc.tensor.matmul(sS[:], ones128[:], ptm[:],
                                     start=st, stop=sp)
                srow = qpool.tile([1, mb], F32R, tag="srow")
                with nc.allow_low_precision(reason="f32r storage is full f32"):
                    nc.vector.reciprocal(srow[:], sS[:])
                sbps = opool.tile([P, mb], F32, tag="u")
                nc.tensor.matmul(sbps[:], ones1[:], srow[:],
                                 start=True, stop=True)
                sinvb = qpool.tile([P, mb], F32, tag="sinvb")
                nc.vector.tensor_copy(sinvb[:], sbps[:])
                nk0 = qpool.tile([P, mb], F32R, tag="nk0")
                nc.vector.tensor_mul(nk0[:], n0[:], sinvb[:])
                nk1 = qpool.tile([P, mb], F32R, tag="nk1")
                nc.vector.tensor_mul(nk1[:], n1[:], sinvb[:])
                t0 = qpool.tile([P, mb], F32R, tag="t0")
                nc.gpsimd.tensor_mul(t0[:], nk0[:], exEhT[0][:, ms])
                t1 = qpool.tile([P, mb], F32R, tag="t1")
                nc.gpsimd.tensor_mul(t1[:], nk1[:], exEhT[1][:, ms])
                feat = [nk0, nk1, t0, t1]
                for oo in range(2):
                    ups = opool.tile([P, mb], F32, tag="u")
                    for dd in range(4):
                        nc.tensor.matmul(
                            ups[:], rdwT_sb[dd][:, oo * P:(oo + 1) * P],
                            feat[dd][:], start=(dd == 0), stop=(dd == 3))
                    # elu(x) = max(x,0) + exp(min(x,0)) - 1,  x = ups + rd_b
                    tmin = qpool.tile([P, mb], F32, tag="tmin")
                    nc.vector.tensor_scalar(tmin[:], ups[:], rdb_sb[oo][:],
                                            0.0, AluOpType.add, AluOpType.min)
                    eneg = qpool.tile([P, mb], F32, tag="eneg")
                    nc.scalar.activation(eneg[:], tmin[:], AF.Exp)
                    tmax = qpool.tile([P, mb], F32, tag="tmax")
                    nc.vector.tensor_scalar(tmax[:], ups[:], rdb_sb[oo][:],
                                            0.0, AluOpType.add, AluOpType.max)
                    res = qpool.tile([P, mb], F32, tag="res")
                    nc.vector.scalar_tensor_tensor(res[:], tmax[:], -1.0,
                                                   eneg[:], AluOpType.add,
                                                   AluOpType.add)
                    nc.sync.dma_start(out=outT[oo * P:(oo + 1) * P, ms],
                                      in_=res[:])
    nc.finalize()
    return nc


_PROGRAM = None


def _get_program():
    global _PROGRAM
    if _PROGRAM is None:
        _PROGRAM = _build()
    return _PROGRAM


def _in_maps(exercise_h, kc_h, adj, W1, E, a, rd_w, rd_b):
    f = np.float32
    a1 = np.ascontiguousarray(a[:D, 0], dtype=f)
    a2 = np.ascontiguousarray(a[D:, 0], dtype=f)
    W1 = np.asarray(W1, dtype=f)
    w1a2 = W1 @ a2
    W1e = np.concatenate([W1, w1a2[:, None],
                          np.zeros((D, 1), f)], axis=1)      # [256, 258]
    w1a1 = (W1 @ a1)[:, None]                                 # [256, 1]
    kcT = np.zeros((2 * P, NKC), dtype=f)
    kcT[:, :2000] = np.asarray(kc_h, dtype=f).T
    Em = np.ascontiguousarray(np.asarray(E, dtype=f))
    rdwT = np.ascontiguousarray(np.asarray(rd_w, dtype=f).T)  # [512, 256]
    rdb = np.asarray(rd_b, dtype=f)[:, None]                  # [256, 1]
    shared = {"kcT": kcT, "W1e": np.ascontiguousarray(W1e),
              "w1a1": np.ascontiguousarray(w1a1), "Em": Em,
              "rdwT": rdwT, "rdb": np.ascontiguousarray(rdb)}
    maps = []
    for c in range(NCORES):
        sl = slice(c * ROWS, (c + 1) * ROWS)
        exT_c = np.zeros((2 * P, M), dtype=f)
        exT_c[:, :ROWS] = np.asarray(exercise_h[sl], dtype=f).T
        adjx = np.asarray(adj[sl], dtype=np.float32).T  # [2000, 1250] of 0/1
        adjT_c = np.zeros((NKC, M), dtype=ml_dtypes.bfloat16)
        for kk in range(KCH):
            rs = slice(kk * P, (kk + 1) * P)
            blk = np.zeros((P, M), dtype=np.float32)
            nreal = max(0, min(2000 - kk * P, P))
            if VARIANTS[kk % 8] == "A":   # multiply-mask chunk: 0/1
                blk[:nreal, :ROWS] = adjx[kk * P:kk * P + nreal]
                blk[:nreal, ROWS:] = 1.0   # pad rows finite
                blk[nreal:, :] = 0.0       # pad kc nodes masked out
            else:                 # logit-fold chunk: 0/-100, pad kc = -100
                blk[:nreal, :ROWS] = (adjx[kk * P:kk * P + nreal] - 1.0) * 100.0
                blk[:nreal, ROWS:] = 0.0
                blk[nreal:, :] = -100.0
            adjT_c[rs] = blk
        del adjx
        maps.append({"exT": exT_c, "adjT": adjT_c, **shared})
    return maps


def kernel(exercise_h, kc_h, adj, W1, E, a, rd_w, rd_b):
    nc = _get_program()
    maps = _in_maps(exercise_h, kc_h, adj, W1, E, a, rd_w, rd_b)
    res = run_bass_kernel_spmd(nc, maps, list(range(NCORES))).results
    out = np.empty((N_E, D), dtype=np.float32)
    for c in range(NCORES):
        out[c * ROWS:(c + 1) * ROWS] = res[c]["outT"][:, :ROWS].T
    return out
